# revision 17
# baseline (speedup 1.0000x reference)
"""Trainium2 Bass kernel for nn_MultiHeadAttention_61778809586301.

Head-sharded across 8 NeuronCores: core `a` computes output row-group `a`
(which, per the reference's faithful-TF recombination, is attention head `a`
across all 8 batches, concatenated batch-major along channels, then Wo+relu).

Per-core device work (all f32):
  - projections q/k/v for all 8 batches, head slice `a` (weights host-sliced,
    inputs host-transposed to feature-major so the contraction dim lands on
    SBUF partitions)
  - causal + key-mask softmax attention, exact additive-mask semantics
  - recombine via sum_c O_c @ Wo[c-slot] with relu + query-mask epilogue

Batches are processed in pairs (c, c+4) packed into SBUF partition halves so
K=64 / M=64 matmuls run as concurrent row/col-tiled pairs on the PE array.
"""
import sys

if "/opt/trn_rl_repo" not in sys.path:
    sys.path.insert(0, "/opt/trn_rl_repo")

import numpy as np

B, S, D, H, DH = 8, 1024, 512, 8, 64
NEG = np.float32(1.0e9)
NPAIR = 4          # batch pairs (p, p+4)
NBLK = S // 128    # 8 sq blocks of 128
KO = D // 128      # 4 contraction chunks of 128

_CACHE: dict = {}
RUN_KWARGS: dict = {}   # extra kwargs for run_bass_kernel_spmd (e.g. trace)
LAST_RESULT = None      # BassKernelResults of the most recent kernel() call


def _build():
    import concourse.mybir as mybir
    import concourse.tile as tile
    from concourse import bacc
    from concourse.masks import make_identity

    f32 = mybir.dt.float32
    nc = bacc.Bacc(
        "TRN2",
        target_bir_lowering=False,
        debug=False,
        enable_asserts=False,
        num_devices=H,
    )

    xt_q = nc.dram_tensor("xt_q", [D, B * S], f32, kind="ExternalInput")
    xt_k = nc.dram_tensor("xt_k", [D, B * S], f32, kind="ExternalInput")
    xt_v = nc.dram_tensor("xt_v", [D, B * S], f32, kind="ExternalInput")
    wq_d = nc.dram_tensor("wq", [D, DH], f32, kind="ExternalInput")
    wk_d = nc.dram_tensor("wk", [D, DH], f32, kind="ExternalInput")
    wv_d = nc.dram_tensor("wv", [D, DH], f32, kind="ExternalInput")
    wo_d = nc.dram_tensor("wo_p", [NPAIR, 128, D], f32, kind="ExternalInput")
    madd_d = nc.dram_tensor("madd", [S, S], f32, kind="ExternalInput")
    kmb_d = nc.dram_tensor("kmb", [128, S], f32, kind="ExternalInput")
    n1t_d = nc.dram_tensor("n1t", [128, NBLK], f32, kind="ExternalInput")
    qm_d = nc.dram_tensor("qmask", [128, NBLK], f32, kind="ExternalInput")
    out_d = nc.dram_tensor("out", [S, D], f32, kind="ExternalOutput")

    with tile.TileContext(nc) as tc:
        with (
            tc.tile_pool(name="fixed", bufs=1) as fixed,
            tc.tile_pool(name="stage", bufs=4) as stage,
            tc.tile_pool(name="proj", bufs=2) as proj,
            tc.tile_pool(name="work", bufs=2) as work,
            tc.tile_pool(name="ptp", bufs=4) as ptp,
            tc.tile_pool(name="stats", bufs=4) as stats,
            tc.tile_pool(name="psA", bufs=4, space="PSUM") as psA,
            tc.tile_pool(name="psB", bufs=2, space="PSUM") as psB,
        ):
            # ---- constants / weights ----
            ident = fixed.tile([128, 128], f32, tag="ident")
            make_identity(nc, ident[:])

            wq_sb = fixed.tile([128, KO, DH], f32, tag="wq")
            wk_sb = fixed.tile([128, KO, DH], f32, tag="wk")
            wv_sb = fixed.tile([128, KO, DH], f32, tag="wv")
            nc.sync.dma_start(wq_sb[:], wq_d.rearrange("(ko ki) m -> ki ko m", ki=128))
            nc.sync.dma_start(wk_sb[:], wk_d.rearrange("(ko ki) m -> ki ko m", ki=128))
            nc.sync.dma_start(wv_sb[:], wv_d.rearrange("(ko ki) m -> ki ko m", ki=128))

            wo_sb = fixed.tile([128, NPAIR, D], f32, tag="wo")
            nc.sync.dma_start(wo_sb[:], wo_d.rearrange("p ki n -> ki p n"))

            qm_sb = fixed.tile([128, NBLK], f32, tag="qm")
            nc.sync.dma_start(qm_sb[:], qm_d[:, :])

            kmb_sb = fixed.tile([128, S], f32, tag="kmb")
            nc.sync.dma_start(kmb_sb[:], kmb_d[:, :])
            n1t_sb = fixed.tile([128, NBLK], f32, tag="n1t")
            nc.sync.dma_start(n1t_sb[:], n1t_d[:, :])
            neg_big = fixed.tile([128, 1], f32, tag="negbig")
            nc.vector.memset(neg_big[:], -1.0e9)

            madd_sb = []
            for i in range(NBLK):
                L = 128 * (i + 1)
                t = fixed.tile([128, L], f32, tag=f"madd{i}")
                nc.sync.dma_start(t[:], madd_d[128 * i:128 * (i + 1), :L])
                madd_sb.append(t)

            # persistent attention outputs, transposed: [dh of c | dh of c+4] x S
            ot_sb = [
                fixed.tile([128, S], f32, tag=f"ot{p}", name=f"ot{p}")
                for p in range(NPAIR)
            ]

            for p in range(NPAIR):
                # ---- projections for batches (p, p+4) ----
                qkv_pair = []
                for name, xt, w_sb in (
                    ("q", xt_q, wq_sb), ("k", xt_k, wk_sb), ("v", xt_v, wv_sb),
                ):
                    pair_t = proj.tile([128, S], f32, tag=f"{name}T")
                    for g in range(2):
                        c = p + 4 * g
                        st = stage.tile([128, KO, S], f32, tag="xstage")
                        nc.sync.dma_start(
                            st[:],
                            xt[:, c * S:(c + 1) * S].rearrange(
                                "(ko ki) s -> ki ko s", ki=128
                            ),
                        )
                        for hh in range(2):
                            ps = psA.tile([128, 512], f32, tag="ps")
                            for ko in range(KO):
                                nc.tensor.matmul(
                                    ps[64 * g:64 * (g + 1), :],
                                    lhsT=w_sb[:, ko, :],
                                    rhs=st[:, ko, 512 * hh:512 * (hh + 1)],
                                    start=(ko == 0),
                                    stop=(ko == KO - 1),
                                )
                            nc.vector.tensor_copy(
                                pair_t[64 * g:64 * (g + 1), 512 * hh:512 * (hh + 1)],
                                ps[64 * g:64 * (g + 1), :],
                            )
                    qkv_pair.append(pair_t)
                qT, kT, vT = qkv_pair

                # ---- masked-V suffix sums: vks[:, i] = sum_{j>=128(i+1), km=1} v[j]
                # (tail aggregate for the full-width softmax virtual column) ----
                vks = proj.tile([128, NBLK], f32, tag="vks")
                bsum = proj.tile([128, NBLK], f32, tag="bsum")
                for b in range(NBLK):
                    ttscr = work.tile([128, 128], f32, tag="ttscr")
                    nc.vector.tensor_tensor(
                        ttscr[:],
                        vT[:, 128 * b:128 * (b + 1)],
                        kmb_sb[:, 128 * b:128 * (b + 1)],
                        mybir.AluOpType.mult,
                    )
                    nc.vector.tensor_reduce(
                        bsum[:, b:b + 1],
                        ttscr[:],
                        axis=mybir.AxisListType.X,
                        op=mybir.AluOpType.add,
                    )
                nc.vector.memset(vks[:, NBLK - 1:NBLK], 0.0)
                for b in range(NBLK - 2, -1, -1):
                    nc.vector.tensor_tensor(
                        vks[:, b:b + 1],
                        vks[:, b + 1:b + 2],
                        bsum[:, b + 1:b + 2],
                        mybir.AluOpType.add,
                    )

                # ---- V back to natural layout [sk, dh] per (block j, g) ----
                vnat = proj.tile([128, NBLK, 2, DH], f32, tag="vnat")
                for g in range(2):
                    for j in range(NBLK):
                        pt = psB.tile([128, 128], f32, tag="pb")
                        nc.tensor.transpose(
                            pt[:, :DH],
                            vT[64 * g:64 * (g + 1), 128 * j:128 * (j + 1)],
                            ident[64 * g:64 * (g + 1), 64 * g:64 * (g + 1)],
                        )
                        nc.vector.tensor_copy(vnat[:, j, g, :], pt[:, :DH])

                # ---- attention per (g, sq-block i) ----
                for g in range(2):
                    for i in range(NBLK):
                        L = 128 * (i + 1)
                        nch = (L + 511) // 512
                        t_sb = work.tile([128, S], f32, tag="t")
                        for n in range(nch):
                            Ln = min(512, L - 512 * n)
                            ps = psA.tile([128, 512], f32, tag="ps")
                            nc.tensor.matmul(
                                ps[:, :Ln],
                                lhsT=qT[64 * g:64 * (g + 1), 128 * i:128 * (i + 1)],
                                rhs=kT[64 * g:64 * (g + 1), 512 * n:512 * n + Ln],
                                start=True,
                                stop=True,
                            )
                            # t = min(sims, mclip): masked entries become the
                            # exact post-mask constants (-1e9/-2e9, matching
                            # jax's f32 absorption of "x - 1e9"), unmasked pass
                            # through (mclip=+FLT_MAX). Bit-exact tie semantics
                            # independent of engine rounding modes.
                            nc.vector.tensor_tensor(
                                t_sb[:, 512 * n:512 * n + Ln],
                                ps[:, :Ln],
                                madd_sb[i][:, 512 * n:512 * n + Ln],
                                mybir.AluOpType.min,
                            )
                        mneg = stats.tile([128, 1], f32, tag="mneg")
                        nc.vector.tensor_reduce(
                            mneg[:],
                            t_sb[:, :L],
                            axis=mybir.AxisListType.X,
                            op=mybir.AluOpType.max,
                            negate=True,
                        )
                        p_sb = work.tile([128, S], f32, tag="p")
                        ssum = stats.tile([128, 1], f32, tag="ssum")
                        nc.scalar.activation(
                            p_sb[:, :L],
                            t_sb[:, :L],
                            mybir.ActivationFunctionType.Exp,
                            bias=mneg[:],
                            scale=1.0,
                            accum_out=ssum[:],
                        )
                        # virtual tail column: weight exp(-1e9 - m) per row
                        # (0 for normal rows; 1 for fully-masked rows), with
                        # n1t tail-tie count folded into the softmax sum.
                        etail = stats.tile([128, 1], f32, tag="etail")
                        nc.scalar.activation(
                            etail[:],
                            neg_big[:],
                            mybir.ActivationFunctionType.Exp,
                            bias=mneg[:],
                            scale=1.0,
                        )
                        etn = stats.tile([128, 1], f32, tag="etn")
                        nc.vector.tensor_tensor(
                            etn[:], etail[:], n1t_sb[:, i:i + 1],
                            mybir.AluOpType.mult,
                        )
                        ssum2 = stats.tile([128, 1], f32, tag="ssum2")
                        nc.vector.tensor_tensor(
                            ssum2[:], ssum[:], etn[:], mybir.AluOpType.add,
                        )
                        rcp = stats.tile([128, 1], f32, tag="rcp")
                        nc.vector.reciprocal(rcp[:], ssum2[:])
                        ptail = stats.tile([128, 1], f32, tag="ptail")
                        nc.vector.tensor_tensor(
                            ptail[:], etail[:], rcp[:], mybir.AluOpType.mult,
                        )
                        ptrep = stats.tile([128, DH], f32, tag="ptrep")
                        nc.vector.tensor_copy(
                            ptrep[:], ptail[:, 0:1].to_broadcast((128, DH)),
                        )
                        pn_sb = work.tile([128, S], f32, tag="pn")
                        nc.scalar.activation(
                            pn_sb[:, :L],
                            p_sb[:, :L],
                            mybir.ActivationFunctionType.Identity,
                            bias=0.0,
                            scale=rcp[:],
                        )
                        # transpose P blocks and accumulate PV into OT
                        po = psB.tile([128, 128], f32, tag="pvo")
                        for j in range(i + 1):
                            ptp_ps = psB.tile([128, 128], f32, tag="pb")
                            nc.tensor.transpose(
                                ptp_ps[:],
                                pn_sb[:, 128 * j:128 * (j + 1)],
                                ident[:],
                            )
                            pt_sb = ptp.tile([128, 128], f32, tag="pt")
                            nc.vector.tensor_copy(pt_sb[:], ptp_ps[:])
                            nc.tensor.matmul(
                                po[64 * g:64 * (g + 1), :],
                                lhsT=vnat[:, j, g, :],
                                rhs=pt_sb[:],
                                start=(j == 0),
                                stop=(j == i),
                            )
                        # tail contribution: ot = po + ptail[sq] * vks[d]
                        gs = slice(64 * g, 64 * (g + 1))
                        btail = psB.tile([128, 128], f32, tag="pb")
                        nc.tensor.matmul(
                            btail[gs, :],
                            lhsT=ptrep[:],
                            rhs=ident[:],
                            start=True,
                            stop=True,
                        )
                        ttl = work.tile([128, 128], f32, tag="ttscr")
                        nc.vector.tensor_tensor(
                            ttl[gs, :],
                            btail[gs, :],
                            vks[gs, i:i + 1].to_broadcast((64, 128)),
                            mybir.AluOpType.mult,
                        )
                        nc.vector.tensor_tensor(
                            ot_sb[p][gs, 128 * i:128 * (i + 1)],
                            po[gs, :],
                            ttl[gs, :],
                            mybir.AluOpType.add,
                        )

            # ---- final projection + relu + query-mask ----
            for i in range(NBLK):
                ps = psA.tile([128, 512], f32, tag="ps")
                for p in range(NPAIR):
                    nc.tensor.matmul(
                        ps[:],
                        lhsT=ot_sb[p][:, 128 * i:128 * (i + 1)],
                        rhs=wo_sb[:, p, :],
                        start=(p == 0),
                        stop=(p == NPAIR - 1),
                    )
                o_sb = work.tile([128, D], f32, tag="osb")
                nc.scalar.activation(
                    o_sb[:],
                    ps[:],
                    mybir.ActivationFunctionType.Relu,
                    bias=0.0,
                    scale=qm_sb[:, i:i + 1],
                )
                nc.sync.dma_start(out_d[128 * i:128 * (i + 1), :], o_sb[:])

    nc.compile()
    return nc


def _get_nc():
    if "nc" not in _CACHE:
        _CACHE["nc"] = _build()
    return _CACHE["nc"]


def _host_prep(query, key, value, query_mask, key_mask, Wq, Wk, Wv, Wo):
    """Build the 8 per-core input maps (numpy only)."""
    inv = np.float32(1.0) / np.sqrt(np.float32(D))

    def tfeat(x):  # (B,S,D) -> feature-major (D, B*S), contiguous
        return np.ascontiguousarray(
            x.reshape(B * S, D).astype(np.float32, copy=False).T
        )

    xq, xk, xv = tfeat(query), tfeat(key), tfeat(value)
    kmf = key_mask.astype(np.float32)
    qmf = query_mask.astype(np.float32)
    causal = np.triu(np.full((S, S), NEG, np.float32), k=1)
    Wqf = Wq.astype(np.float32, copy=False)
    Wkf = Wk.astype(np.float32, copy=False)
    Wvf = Wv.astype(np.float32, copy=False)
    Wof = Wo.astype(np.float32, copy=False)

    wo_p = np.stack(
        [
            np.concatenate(
                [Wof[p * DH:(p + 1) * DH, :], Wof[(p + 4) * DH:(p + 5) * DH, :]],
                axis=0,
            )
            for p in range(NPAIR)
        ]
    )  # (4, 128, 512)

    FBIG = np.finfo(np.float32).max
    in_maps = []
    for a in range(H):
        madd_sum = (causal + NEG * (1.0 - kmf[a])[None, :]).astype(np.float32)
        # min-clip tile: exact masked values where masked, +FLT_MAX where not
        madd = np.where(madd_sum > 0, -madd_sum, FBIG).astype(np.float32)
        in_maps.append(
            {
                "xt_q": xq,
                "xt_k": xk,
                "xt_v": xv,
                "wq": np.ascontiguousarray(
                    Wqf[:, a * DH:(a + 1) * DH] * inv
                ).astype(np.float32),
                "wk": np.ascontiguousarray(Wkf[:, a * DH:(a + 1) * DH]),
                "wv": np.ascontiguousarray(Wvf[:, a * DH:(a + 1) * DH]),
                "wo_p": wo_p,
                "madd": madd,
                "kmb": np.ascontiguousarray(
                    np.broadcast_to(kmf[a][None, :], (128, S))
                ),
                "n1t": np.ascontiguousarray(
                    np.broadcast_to(
                        np.array(
                            [kmf[a, 128 * (i + 1):].sum() for i in range(NBLK)],
                            np.float32,
                        )[None, :],
                        (128, NBLK),
                    )
                ),
                "qmask": np.ascontiguousarray(
                    qmf[a].reshape(NBLK, 128).T
                ),  # [p, blk] = qm[a, 128*blk + p]
                "out": None,  # placeholder removed below
            }
        )
        del in_maps[-1]["out"]
    return in_maps


def kernel(**inputs) -> np.ndarray:
    from concourse.bass_utils import run_bass_kernel_spmd

    nc = _get_nc()
    in_maps = _host_prep(
        np.asarray(inputs["query"]),
        np.asarray(inputs["key"]),
        np.asarray(inputs["value"]),
        np.asarray(inputs["query_mask"]),
        np.asarray(inputs["key_mask"]),
        np.asarray(inputs["Wq"]),
        np.asarray(inputs["Wk"]),
        np.asarray(inputs["Wv"]),
        np.asarray(inputs["Wo"]),
    )
    res = run_bass_kernel_spmd(nc, in_maps, core_ids=list(range(H)), **RUN_KWARGS)
    global LAST_RESULT
    LAST_RESULT = res
    return np.stack([res.results[a]["out"] for a in range(H)])
